# revision 12
# baseline (speedup 1.0000x reference)
"""GA2 MaxPool2d (K=2, stride 2) as a Bass/Tile kernel on 8 Trainium2 cores.

Sharding: pure data parallel over batch (16 -> 2 per core). Per core the
input shard (2, 256, 128, 128) is viewed as [p=128, k=4, h=128, w=128]
where p = b_local*64 + channel_group and k is the GA2 component
(channel = 4*group + k), so the whole pool is per-partition work.

Math must match the jax reference bit-exactly so near-tie argmax picks the
same window element (verified on the fixed input: the reference sums
component squares sequentially): mag = ((v0^2 + v1^2) + v2^2) + v3^2 in
fp32 in that exact order. Winner = first window position (row-major) whose
mag equals the window max; selection is equality masks + reverse-priority
predicated copies, which is exact argmax-first-occurrence, ties included.

Engine split per 8-row chunk (DMA-bound target ~117us/core):
  ACT   : 4 squares, de-interleave positions 0..2, seed output with pos 3
  DVE   : first mag add, 3 int32-bitcast equality masks, 3 copy_predicated
  GPSIMD: remaining 2 sequential mag adds, 3 window maxes
  SYNC  : HBM DMAs (in: 2MB/chunk, out: 0.5MB/chunk)
"""

import numpy as np

B, C4, H, W = 16, 256, 128, 128
N_CORES = 8
BPC = B // N_CORES          # batches per core
P = 128                     # partitions = BPC * 64 channel groups
KC = 4                      # GA2 components per group
R = 8                       # input rows per chunk
RO = R // 2                 # output rows per chunk
WO = W // 2
Q = RO * WO                 # pooled elems per chunk per partition
HO = H // 2
NCHUNK = H // R
YPAD = 4                    # pad yt free stride so AP views don't coalesce

_CACHE = {}


def build_program():
    """Build (once) the Bass module for one core's shard."""
    if "nc" in _CACHE:
        return _CACHE["nc"]

    from contextlib import ExitStack

    import concourse.bacc as bacc
    import concourse.mybir as mybir
    import concourse.tile as tile

    f32 = mybir.dt.float32
    i32 = mybir.dt.int32
    Alu = mybir.AluOpType

    # Bacc (not bare Bass): its compile() pass nop-splits multi-sem waits,
    # which the TT ISA structs can't encode.
    nc = bacc.Bacc("TRN2", target_bir_lowering=False, debug=False)
    x = nc.dram_tensor("x", [P, KC, H, W], f32, kind="ExternalInput").ap()
    y = nc.dram_tensor("y", [P, KC, HO * WO], f32, kind="ExternalOutput").ap()

    POS = ((0, 0), (0, 1), (1, 0), (1, 1))  # row-major window positions

    with tile.TileContext(nc) as tc, ExitStack() as ctx:
        tin_pool = ctx.enter_context(tc.tile_pool(name="tin", bufs=3))
        sq_pool = ctx.enter_context(tc.tile_pool(name="sq", bufs=2))
        sel_pool = ctx.enter_context(tc.tile_pool(name="sel", bufs=2))
        dei_pool = ctx.enter_context(tc.tile_pool(name="dei", bufs=2))
        out_pool = ctx.enter_context(tc.tile_pool(name="out", bufs=3))

        for c in range(NCHUNK):
            h0 = c * R
            tin = tin_pool.tile([P, KC, R, W], f32, tag="tin")
            nc.sync.dma_start(out=tin[:], in_=x[:, :, h0 : h0 + R, :])

            # squares on ACT; mag accumulated sequentially (reference order):
            # mag = ((s0 + s1) + s2) + s3, adds split DVE/gpsimd, in place.
            sq = sq_pool.tile([P, KC, R, W], f32, tag="sq")
            for k in range(KC):
                nc.scalar.square(sq[:, k], tin[:, k])
            mag = sq[:, 0]
            nc.vector.tensor_tensor(mag, mag, sq[:, 1], Alu.add)
            nc.gpsimd.tensor_tensor(mag, mag, sq[:, 2], Alu.add)
            nc.gpsimd.tensor_tensor(mag, mag, sq[:, 3], Alu.add)

            # per-window-position views of mag
            mp = [mag[:, dh::2, dw::2] for (dh, dw) in POS]

            # maxes on DVE: gpsimd TT can't read strided (stride-2) views
            t01 = sel_pool.tile([P, RO, WO], f32, tag="t01")
            nc.vector.tensor_tensor(t01[:], mp[0], mp[1], Alu.max)
            t23 = sel_pool.tile([P, RO, WO], f32, tag="t23")
            nc.vector.tensor_tensor(t23[:], mp[2], mp[3], Alu.max)
            best = sel_pool.tile([P, RO, WO], f32, tag="best")
            nc.vector.tensor_tensor(best[:], t01[:], t23[:], Alu.max)

            # equality masks: compare as fp32 (the DVE ALU is fp32 internal, so
            # int32 bitcast compares round and collide; fp32 == fp32 is exact),
            # int32 output dtype for the CopyPredicated mask requirement
            preds = sel_pool.tile([P, 3, RO, WO], i32, tag="preds")
            for p in range(3):
                nc.vector.tensor_tensor(preds[:, p], mp[p], best[:], Alu.is_equal)

            # de-interleave positions 0..2 into contiguous blocks (ACT)
            tin_d = dei_pool.tile([P, KC, 3, Q], f32, tag="tind")
            for p in range(3):
                dh, dw = POS[p]
                nc.scalar.copy(tin_d[:, :, p], tin[:, :, dh::2, dw::2])

            # seed with position 3, then overwrite 2, 1, 0 (first occurrence wins)
            yt = out_pool.tile([P, KC, Q + YPAD], f32, tag="yt")
            ytv = yt[:, :, :Q]
            nc.scalar.copy(ytv, tin[:, :, 1::2, 1::2])
            for p in (2, 1, 0):
                mask = (
                    preds[:, p]
                    .rearrange("p a b -> p (a b)")
                    .unsqueeze(1)
                    .broadcast_to([P, KC, Q])
                )
                nc.vector.copy_predicated(ytv, mask, tin_d[:, :, p])

            nc.sync.dma_start(out=y[:, :, c * Q : (c + 1) * Q], in_=ytv)

    nc.compile()
    _CACHE["nc"] = nc
    return nc


def _shard_inputs(x):
    xs = x.reshape(N_CORES, BPC, C4, H, W)
    return [
        {"x": np.ascontiguousarray(xs[i]).reshape(P, KC, H, W)}
        for i in range(N_CORES)
    ]


def _unshard(outs):
    full = np.empty((B, C4, HO, WO), dtype=np.float32)
    for i, yi in enumerate(outs):
        full[i * BPC : (i + 1) * BPC] = np.asarray(yi).reshape(BPC, C4, HO, WO)
    return full


def run(x, trace=False):
    """Run on all 8 cores; returns (out_full, exec_time_ns_or_None)."""
    from concourse.bass_utils import run_bass_kernel_spmd

    nc = build_program()
    in_maps = _shard_inputs(x)
    res = run_bass_kernel_spmd(
        nc, in_maps, core_ids=list(range(N_CORES)), trace=trace
    )
    out = _unshard([res.results[i]["y"] for i in range(N_CORES)])
    return out, res.exec_time_ns


def kernel(x):
    x = np.ascontiguousarray(np.asarray(x, dtype=np.float32))
    assert x.shape == (B, C4, H, W), x.shape
    out, _ = run(x, trace=False)
    return out
